# revision 1
# baseline (speedup 1.0000x reference)
"""Trainium2 Bass kernel for nn_Ensembler (nms_detection).

Contract: kernel(**inputs) takes the FULL unsharded inputs
(voxel_logits [3,64,128,128,32] f32, query_logits [3,1,64,21] f32,
sem_prob_dense [21,128,128,32] f32) and returns the FULL output
[64,128,128,32] f32.

Strategy: shard the voxel grids over the flattened voxel dimension
N = X*Y*Z across 8 NeuronCores (each core owns a contiguous slice of
N).  The QxQ IoU statistics are computed as per-shard 0/1-mask GEMMs
(fp8 DoubleRow on the tensor engine) reduced with a tiny AllReduce;
the argmax / matching / merge / keep steps are then replicated on
every core, and the merge + keep + occupancy masking are
embarrassingly parallel over the local N slice.  The data-dependent
row gather aux_v[aux_idx] is realized as indirect DMAs that read the
aux logits from DRAM with device-computed row indices.

Numerical notes:
 - all mask decisions are computed from logit signs (exact): the
   iteration-2 anchor mask uses (sig(x0)+sig(x1))/2 > 0.5 <=>
   x0 + x1 > 0, avoiding sigmoid-LUT error in the decision path.
 - sigmoid LUT (ScalarE) max abs err ~3.6e-6 affects output values
   only.

Layouts per core (NS = 65536 voxels):
 - "n-layout": [128 part, ...] with n = p*512 + j (partition-major).
 - "q-layout": [128 part = (qb, q), T cols]: chunk ci covers
   n in [ci*2T, ci*2T+2T); rows 0:64 hold q for the first T, rows
   64:128 the second T.
 - L0 is read ONCE into a persistent q-layout SBUF tile that is
   overwritten in place by the merged anchor (pass B) and consumed by
   pass C.  Masks travel through DRAM as fp8 to switch layouts.
"""

import numpy as np

S = 3
Q = 64
X, Y, Z = 128, 128, 32
N = X * Y * Z           # 524288
C_SEM = 21
NCORES = 8
NS = N // NCORES        # 65536 voxels per core
JP = NS // 128          # 512 contiguous voxels per partition (n-layout)
T = 1024                # q-layout chunk free size
NCH = NS // (2 * T)     # 32 q-layout chunks
QC = 4                  # q rows per n-layout read chunk

_compiled = None


def _register_custom_dve_ops():
    """Register two fused DVE ops at runtime (halves the DVE op count on
    the blend/mask hot paths).  Purely additive registration in the
    concourse dve_ops tables; rows stay within the 5-bit byte-36 field."""
    import concourse.dve_ops as dve_ops
    from concourse.dve_ops import DveOp
    from concourse.dve_spec import (Spec, Src0, Src1, C0, C1, Zero, lower,
                                    _has_src1)
    from concourse.dve_uop import DveOpSpec

    if "ANT_BLEND2_K" in dve_ops._SUB_OPCODE_FOR_NAME:
        by = {op.name: op for op in dve_ops.OPS}
        return by["ANT_BLEND2_K"], by["ANT_MASKGT_K"]

    def make(name, spec):
        row = dve_ops._CUSTOM_DVE_ROW_BASE + len(dve_ops.OPS)
        assert row < 0x20
        dve_ops._SUB_OPCODE_FOR_NAME[name] = row
        shas = {}
        for ver in ("v3", "v4"):
            try:
                uops = lower(spec, ver=ver)
                shas[ver] = DveOpSpec(name=name, opcode=row, uops=uops,
                                      rd1_en=_has_src1(spec)).sha(ver)
            except Exception:
                pass
        op = DveOp(name, spec, subdim=False, uops_sha=shas)
        dve_ops.OPS.append(op)
        dve_ops.CUSTOM_DVE_SPECS[name] = spec
        return op

    blend2 = make("ANT_BLEND2_K", Spec(
        body=Src0 * C0 + Src1 * C1,
        reference=lambda in0, in1, s0, s1, imm2: (
            in0.astype(np.float32) * s0 + in1 * s1).astype(np.float32),
    ))
    maskgt = make("ANT_MASKGT_K", Spec(
        body=Zero < (Src0 + Src1 * C0),
        reference=lambda in0, in1, s0, s1, imm2: (
            (in0.astype(np.float32) + in1 * s0) > 0).astype(np.float32),
    ))
    return blend2, maskgt


def _build_program(phases=("A", "AR1", "B", "G2", "AR2", "C"), real_cc=True,
                   loop_k=None):
    import dataclasses
    import concourse.bass as bass
    import concourse.bacc as bacc
    import concourse.mybir as mybir
    import concourse.tile as tile

    phases = set(phases)
    dt = mybir.dt
    Alu = mybir.AluOpType
    Act = mybir.ActivationFunctionType
    DR = mybir.MatmulPerfMode.DoubleRow

    BLEND2, MASKGT = _register_custom_dve_ops()

    def dram_view(ap, pattern, offset_elems):
        """Raw [step,count] (element units) view of a DRAM tensor AP."""
        return dataclasses.replace(ap, ap=[list(p) for p in pattern],
                                   offset=offset_elems)

    nc = bacc.Bacc("TRN2", target_bir_lowering=False, debug=False,
                   num_devices=NCORES)

    l0 = nc.dram_tensor("l0", [Q, NS], dt.float32, kind="ExternalInput").ap()
    l1 = nc.dram_tensor("l1", [Q, NS], dt.float32, kind="ExternalInput").ap()
    l2 = nc.dram_tensor("l2", [Q, NS], dt.float32, kind="ExternalInput").ap()
    sem = nc.dram_tensor("sem", [C_SEM, NS], dt.float32,
                         kind="ExternalInput").ap()
    revcnt = nc.dram_tensor("revcnt", [Q, Q], dt.float32,
                            kind="ExternalInput").ap()
    iotap = nc.dram_tensor("iotap", [128, 1], dt.float32,
                           kind="ExternalInput").ap()
    out = nc.dram_tensor("out", [Q, NS], dt.float32,
                         kind="ExternalOutput").ap()

    import contextlib

    with tile.TileContext(nc) as tc:
        with (tc.For_i(0, loop_k, 1) if loop_k else
              contextlib.nullcontext()):
            _body(nc, tc, phases, real_cc, dram_view,
                  (l0, l1, l2, sem, revcnt, iotap, out), (BLEND2, MASKGT),
                  mybir)
    nc.compile()
    return nc


def _body(nc, tc, phases, real_cc, dram_view, tensors, custom_ops, mybir):
    import dataclasses
    import concourse.bass as bass

    dt = mybir.dt
    Alu = mybir.AluOpType
    Act = mybir.ActivationFunctionType
    DR = mybir.MatmulPerfMode.DoubleRow
    l0, l1, l2, sem, revcnt, iotap, out = tensors
    BLEND2, MASKGT = custom_ops

    if True:
        with tc.tile_pool(name="dram", bufs=1, space="DRAM") as dramp, \
             tc.tile_pool(name="psum", bufs=1, space="PSUM") as psump, \
             tc.tile_pool(name="stats", bufs=1) as stp:

            # ---- DRAM scratch ----------------------------------------
            m0_dram = dramp.tile([Q + 1, NS], dt.float8e4)
            ma2_dram = dramp.tile([Q + 1, NS], dt.float8e4)
            occ_dram = dramp.tile([1, NS], dt.float8e4)
            cc_in1 = dramp.tile([Q + 1, Q + 1], dt.float32)
            cc_out1 = dramp.tile([Q + 1, Q + 1], dt.float32)
            cc_in2 = dramp.tile([Q + 1, Q + 1], dt.float32)
            cc_out2 = dramp.tile([Q + 1, Q + 1], dt.float32)
            pack1_dram = dramp.tile([Q, 3], dt.float32)
            pack2_dram = dramp.tile([Q, 3], dt.float32)

            # ---- small persistent stat tiles -------------------------
            revc = stp.tile([Q, Q], dt.float32)
            nc.sync.dma_start(revc[:], revcnt[:])
            iou_a1 = stp.tile([Q, 1], dt.float32)
            iou_a2 = stp.tile([Q, 1], dt.float32)
            iotp = stp.tile([128, 1], dt.float32)
            nc.sync.dma_start(iotp[:], iotap[:])
            bd1 = stp.tile([128, 128], dt.float32)
            bd2 = stp.tile([128, 128], dt.float32)
            idxb_dram = dramp.tile([1, 2 * Q], dt.float32)
            idxb_dram2 = dramp.tile([1, 2 * Q], dt.float32)
            cb_pp = stp.tile([128, 3], dt.float32)   # [cb, matched1, 1-cb]
            c3k_pp = stp.tile([128, 3], dt.float32)  # [c3, keep, 1-c3]

            g1_ps = psump.tile([Q + 1, Q + 1], dt.float32)
            g2_ps = psump.tile([Q + 1, Q + 1], dt.float32)

            # big persistent region: holds L0 logits, then anchor2 in
            # place.  Split into 8 tiles so unit-level deps stay fine-
            # grained (one tile = 8 blend units of 512 cols).
            with tc.tile_pool(name="bigp", bufs=1) as bigp:
                l0q_tiles = []
                for b in range(8):
                    lt = bigp.tile([128, NS // 16], dt.float32,
                                   name=f"l0q_{b}")
                    l0q_tiles.append(lt)
                    for qb in range(2):
                        eng = nc.sync if (b + qb) % 2 == 0 else nc.scalar
                        eng.dma_start(
                            lt[qb * Q:(qb + 1) * Q, :],
                            dram_view(l0,
                                      [[NS, Q], [2 * T, 4], [1, T]],
                                      b * 4 * 2 * T + qb * T))

                def l0q_slice(u):
                    # unit u covers global cols [u*512, (u+1)*512)
                    ti, off = divmod(u * 512, NS // 16)
                    return l0q_tiles[ti][:, off:off + 512]

                # =====================================================
                # PASS A: m0 masks -> DRAM roundtrip; m1 (SBUF) -> G1;
                #         m2 masks kept in SBUF for G2
                # =====================================================
                with tc.tile_pool(name="m0p", bufs=1) as pa:
                    ones_c = pa.tile([128, JP], dt.float8e4)
                    nc.vector.memset(ones_c[:], 1.0)
                    nc.scalar.dma_start(
                        dram_view(m0_dram, [[JP, 128], [1, JP]], Q * NS),
                        ones_c[:])
                    # m0 masks from the q-layout L0 tiles -> m0_dram
                    for grp in range(8):
                        m0c = pa.tile([128, 4 * T], dt.float8e4, tag="m0c",
                                      bufs=2)
                        nc.vector.tensor_scalar(
                            m0c[:], l0q_tiles[grp][:], 0.0, None,
                            op0=Alu.is_gt)
                        for qb in range(2):
                            weng = nc.scalar if (grp + qb) % 2 == 0 else nc.sync
                            weng.dma_start(
                                dram_view(m0_dram,
                                          [[NS, Q], [2 * T, 4], [1, T]],
                                          grp * 8 * T + qb * T),
                                m0c[qb * Q:(qb + 1) * Q, :])
                    # m1 masks: n-layout direct to SBUF (j-major + ones col)
                    with tc.tile_pool(name="m1p", bufs=1) as pm1:
                        m1_sb = pm1.tile([128, JP, Q + 1], dt.float8e4)
                        nc.vector.memset(m1_sb[:, :, Q], 1.0)
                        for qc in range(Q // QC):
                            lc = pm1.tile([128, QC, JP], dt.float32,
                                          tag="ldchunk", bufs=2)
                            src = dram_view(l1,
                                            [[JP, 128], [NS, QC], [1, JP]],
                                            qc * QC * NS)
                            ldeng = nc.sync if qc % 2 == 0 else nc.scalar
                            ldeng.dma_start(lc[:], src)
                            nc.vector.tensor_scalar(
                                m1_sb[:, :, qc * QC:(qc + 1) * QC],
                                lc[:].rearrange("p q j -> p j q"), 0.0,
                                None, op0=Alu.is_gt)
                        # G1 GEMM: m0 readback (j-halves) x m1_sb, DoubleRow
                        for h in range(2):
                            m0t = pm1.tile([128, Q + 1, JP // 2],
                                           dt.float8e4, tag="m0t", bufs=1)
                            nc.sync.dma_start(
                                m0t[:],
                                dram_view(
                                    m0_dram,
                                    [[JP, 128], [NS, Q + 1], [1, JP // 2]],
                                    h * (JP // 2)))
                            for j in range(JP // 2):
                                gj = h * (JP // 2) + j
                                nc.tensor.matmul(
                                    g1_ps[:], lhsT=m0t[:, :, j],
                                    rhs=m1_sb[:, gj, :],
                                    start=(gj == 0), stop=(gj == JP - 1))

                # m2 masks: n-layout direct to SBUF, persists through G2
                pm2 = tc.alloc_tile_pool(name="m2p", bufs=1)
                m2_sb = pm2.tile([128, JP, Q + 1], dt.float8e4)
                nc.vector.memset(m2_sb[:, :, Q], 1.0)
                with tc.tile_pool(name="m2fill", bufs=1) as pmf:
                    for qc in range(Q // QC):
                        lc2 = pmf.tile([128, QC, JP], dt.float32,
                                       tag="ld2chunk", bufs=2)
                        src = dram_view(l2, [[JP, 128], [NS, QC], [1, JP]],
                                        qc * QC * NS)
                        ldeng = nc.scalar if qc % 2 == 0 else nc.sync
                        ldeng.dma_start(lc2[:], src)
                        nc.vector.tensor_scalar(
                            m2_sb[:, :, qc * QC:(qc + 1) * QC],
                            lc2[:].rearrange("p q j -> p j q"), 0.0,
                            None, op0=Alu.is_gt)

                # ---- shared stats machinery --------------------------
                def stats_round(g_ps, cc_in, cc_out, iou_a, bd, idx_dram):
                    sfx = cc_in.name
                    gs = stp.tile([Q + 1, Q + 1], dt.float32,
                                  name=f"gs_{sfx}")
                    nc.vector.tensor_copy(gs[:], g_ps[:])
                    nc.sync.dma_start(cc_in[:], gs[:])
                    if real_cc:
                        nc.gpsimd.collective_compute(
                            "AllReduce", Alu.add,
                            replica_groups=[list(range(NCORES))],
                            ins=[cc_in.opt()], outs=[cc_out.opt()])
                    else:
                        nc.sync.dma_start(cc_out[:], cc_in[:])
                    gr = stp.tile([Q + 1, Q + 1], dt.float32,
                                  name=f"gr_{sfx}")
                    nc.sync.dma_start(gr[:], cc_out[:])
                    sbb = stp.tile([Q, Q], dt.float32, name=f"sbb_{sfx}")
                    row = cc_out[Q:Q + 1, 0:Q]
                    nc.sync.dma_start(
                        sbb[:], dataclasses.replace(
                            row, ap=[[0, Q]] + [list(p) for p in row.ap[1:]]))
                    inter = gr[0:Q, 0:Q]
                    sa = gr[0:Q, Q:Q + 1]
                    u = stp.tile([Q, Q], dt.float32, name=f"u_{sfx}")
                    nc.vector.tensor_scalar(u[:], inter, sa, None,
                                            op0=Alu.subtract)
                    nc.vector.tensor_tensor(u[:], sbb[:], u[:],
                                            op=Alu.subtract)
                    nc.vector.tensor_scalar(u[:], u[:], 1.0, None,
                                            op0=Alu.max)
                    nc.vector.reciprocal(u[:], u[:])
                    iou = stp.tile([Q, Q], dt.float32, name=f"iou_{sfx}")
                    nc.vector.tensor_tensor(iou[:], inter, u[:], op=Alu.mult)
                    nc.vector.tensor_reduce(iou_a[:], iou[:],
                                            axis=mybir.AxisListType.X,
                                            op=Alu.max)
                    matched = stp.tile([Q, 1], dt.float32, name=f"mt_{sfx}")
                    nc.vector.tensor_scalar(matched[:], iou_a[:], 0.2, None,
                                            op0=Alu.is_gt)
                    eq = stp.tile([Q, Q], dt.float32, name=f"eq_{sfx}")
                    nc.vector.tensor_scalar(eq[:], iou[:], iou_a[:, 0:1],
                                            None, op0=Alu.is_equal)
                    nc.vector.tensor_tensor(eq[:], eq[:], revc[:],
                                            op=Alu.mult)
                    sm = stp.tile([Q, 1], dt.float32, name=f"sm_{sfx}")
                    nc.vector.tensor_reduce(sm[:], eq[:],
                                            axis=mybir.AxisListType.X,
                                            op=Alu.max)
                    nc.vector.tensor_scalar(sm[:], sm[:], -1.0, float(Q),
                                            op0=Alu.mult, op1=Alu.add)
                    # block-diagonal one-hot gather matrix:
                    # bd[p, m] = (idx[m % 64] + 64*(m//64) == p)
                    pkx = stp.tile([Q, 2], dt.float32, name=f"pkx_{sfx}")
                    nc.vector.tensor_copy(pkx[:, 0:1], sm[:])
                    nc.vector.tensor_scalar(pkx[:, 1:2], sm[:], 64.0, None,
                                            op0=Alu.add)
                    nc.sync.dma_start(
                        dram_view(idx_dram, [[1, Q], [Q, 2]], 0), pkx[:])
                    idxrow = stp.tile([128, 128], dt.float32,
                                      name=f"idxrow_{sfx}")
                    nc.sync.dma_start(
                        idxrow[:],
                        dram_view(idx_dram, [[0, 128], [1, 128]], 0))
                    nc.vector.tensor_scalar(bd[:], idxrow[:], iotp[:, 0:1],
                                            None, op0=Alu.is_equal)
                    return matched

                if "AR1" in phases:
                    matched1 = stats_round(g1_ps, cc_in1, cc_out1, iou_a1,
                                           bd1, idxb_dram)
                    cb64 = stp.tile([Q, 3], dt.float32)
                    nc.vector.tensor_scalar(cb64[:, 0:1], matched1[:], 0.5,
                                            None, op0=Alu.mult)
                    nc.vector.tensor_copy(cb64[:, 1:2], matched1[:])
                    nc.vector.tensor_scalar(cb64[:, 2:3], matched1[:], -0.5,
                                            1.0, op0=Alu.mult, op1=Alu.add)
                    nc.sync.dma_start(pack1_dram[:], cb64[:])
                    nc.sync.dma_start(
                        cb_pp[:],
                        dram_view(pack1_dram, [[0, 2], [3, Q], [1, 3]], 0))

                # =====================================================
                # PASS B: anchor2 blend in place + ma2 mask; G2 GEMM
                # =====================================================
                if "B" in phases:
                    with tc.tile_pool(name="blend", bufs=1) as pb:
                        ones_r = pb.tile([128, JP], dt.float8e4)
                        nc.vector.memset(ones_r[:], 1.0)
                        nc.scalar.dma_start(
                            dram_view(ma2_dram, [[JP, 128], [1, JP]],
                                      Q * NS),
                            ones_r[:])
                        for u in range(NS // 1024):   # 512-wide units
                            ci, hh = u // 2, u % 2
                            sl = l0q_slice(u)
                            l1c = pb.tile([128, 512], dt.float32,
                                          tag="l1c", bufs=4)
                            ldeng = nc.sync if u % 2 == 0 else nc.scalar
                            ldeng.dma_start(
                                l1c[:],
                                dram_view(l1,
                                          [[T, 2], [NS, Q], [1, 512]],
                                          ci * 2 * T + hh * 512))
                            # gather logits on PE: lg = blockdiag(sel1) @ l1c
                            lg = psump.tile([128, 512], dt.float32,
                                            tag="gps", bufs=2,
                                            name=f"lg_{u}")
                            nc.tensor.matmul(lg[:], lhsT=bd1[:], rhs=l1c[:],
                                             start=True, stop=True)
                            # exact mask (l0 + matched1*l1g) > 0 (logits!)
                            if u % 16 == 0:
                                ma2st = pb.tile([128, 8 * T], dt.float8e4,
                                                tag="ma2st", bufs=2)
                            nc.vector._custom_dve(
                                MASKGT,
                                out=ma2st[:, (u % 16) * 512:
                                          (u % 16 + 1) * 512],
                                in0=sl, in1=lg[:], s0=cb_pp[:, 1:2])
                            if u % 16 == 15:
                                grp = u // 16
                                for qb in range(2):
                                    weng = (nc.scalar if (grp + qb) % 2 == 0
                                            else nc.sync)
                                    weng.dma_start(
                                        dram_view(
                                            ma2_dram,
                                            [[NS, Q], [2 * T, 8], [1, T]],
                                            grp * 16 * T + qb * T),
                                        ma2st[qb * Q:(qb + 1) * Q, :])
                            p0c = pb.tile([128, 512], dt.float32, tag="p0c",
                                          bufs=2)
                            nc.scalar.activation(p0c[:], sl, Act.Sigmoid)
                            p1g = pb.tile([128, 512], dt.float32, tag="p1g",
                                          bufs=2)
                            nc.scalar.activation(p1g[:], lg[:], Act.Sigmoid)
                            # anchor2 = (1-cb)*p0 + cb*p1g, in place
                            nc.vector._custom_dve(
                                BLEND2, out=sl, in0=p0c[:], in1=p1g[:],
                                s0=cb_pp[:, 2:3], s1=cb_pp[:, 0:1])

                    if "G2" in phases:
                        with tc.tile_pool(name="g2", bufs=1) as pg:
                            ma2t = pg.tile([128, Q + 1, JP], dt.float8e4)
                            for g in range(8):
                                ps = slice(g * 16, (g + 1) * 16)
                                eng = nc.sync if g % 2 == 0 else nc.scalar
                                eng.dma_start(
                                    ma2t[ps, :, :],
                                    dram_view(
                                        ma2_dram,
                                        [[JP, 16], [NS, Q + 1], [1, JP]],
                                        g * 16 * JP))
                            for j in range(JP):
                                nc.tensor.matmul(
                                    g2_ps[:], lhsT=ma2t[:, :, j],
                                    rhs=m2_sb[:, j, :],
                                    start=(j == 0), stop=(j == JP - 1))
                    pm2.release()

                    if "AR2" in phases:
                        matched2 = stats_round(g2_ps, cc_in2, cc_out2,
                                               iou_a2, bd2, idxb_dram2)
                        pk = stp.tile([Q, 3], dt.float32)
                        nc.vector.tensor_scalar(pk[:, 0:1], matched2[:],
                                                1.0 / 3.0, None,
                                                op0=Alu.mult)
                        nc.vector.tensor_scalar(pk[:, 2:3], matched2[:],
                                                -1.0 / 3.0, 1.0,
                                                op0=Alu.mult, op1=Alu.add)
                        t64 = stp.tile([Q, 1], dt.float32)
                        nc.vector.tensor_tensor(t64[:], iou_a1[:],
                                                iou_a2[:], op=Alu.add)
                        nc.vector.tensor_scalar(pk[:, 1:2], t64[:], 0.5,
                                                0.2, op0=Alu.mult,
                                                op1=Alu.is_gt)
                        nc.sync.dma_start(pack2_dram[:], pk[:])
                        nc.sync.dma_start(
                            c3k_pp[:],
                            dram_view(pack2_dram, [[0, 2], [3, Q], [1, 3]],
                                      0))

                    # =================================================
                    # PASS C: final merge + keep + occupancy -> out
                    # =================================================
                    if "C" in phases:
                        with tc.tile_pool(name="passc", bufs=1) as pc:
                            # occupancy (overlaps the AR2 window):
                            # occ[n] = (max_{c>=1} sem[c,n] > sem[0,n])
                            sem0 = pc.tile([128, JP], dt.float32)
                            nc.sync.dma_start(
                                sem0[:],
                                dram_view(sem, [[JP, 128], [1, JP]], 0))
                            mx = pc.tile([128, JP], dt.float32)
                            nc.sync.dma_start(
                                mx[:],
                                dram_view(sem, [[JP, 128], [1, JP]], NS))
                            for g0 in range(2, C_SEM, 5):
                                rows = min(5, C_SEM - g0)
                                semc = pc.tile([128, 5, JP], dt.float32,
                                               tag="semc", bufs=1,
                                               name=f"semg{g0}")
                                nc.scalar.dma_start(
                                    semc[:, :rows, :],
                                    dram_view(sem,
                                              [[JP, 128], [NS, rows],
                                               [1, JP]],
                                              g0 * NS))
                                for k in range(rows):
                                    nc.vector.tensor_tensor(
                                        mx[:], mx[:], semc[:, k, :],
                                        op=Alu.max)
                            occ_n = pc.tile([128, JP], dt.float8e4)
                            nc.vector.tensor_tensor(occ_n[:], mx[:],
                                                    sem0[:], op=Alu.is_gt)
                            nc.sync.dma_start(
                                dram_view(occ_dram, [[JP, 128], [1, JP]],
                                          0),
                                occ_n[:])
                            occ_all = pc.tile([128, NS // 2], dt.float8e4)
                            for qb in range(2):
                                nc.scalar.dma_start(
                                    occ_all[qb * Q:(qb + 1) * Q, :],
                                    dram_view(
                                        occ_dram,
                                        [[0, Q], [2 * T, NCH], [1, T]],
                                        qb * T))
                            for u in range(NS // 1024):
                                ci, hh = u // 2, u % 2
                                a2s = l0q_slice(u)
                                l2c = pc.tile([128, 512], dt.float32,
                                              tag="l2c", bufs=4)
                                ldeng = nc.sync if u % 2 == 0 else nc.scalar
                                ldeng.dma_start(
                                    l2c[:],
                                    dram_view(l2,
                                              [[T, 2], [NS, Q], [1, 512]],
                                              ci * 2 * T + hh * 512))
                                lg2 = psump.tile([128, 512], dt.float32,
                                                 tag="gps", bufs=2,
                                                 name=f"lg2_{u}")
                                nc.tensor.matmul(lg2[:], lhsT=bd2[:],
                                                 rhs=l2c[:],
                                                 start=True, stop=True)
                                p2g = pc.tile([128, 512], dt.float32,
                                              tag="p2g", bufs=2)
                                nc.scalar.activation(p2g[:], lg2[:],
                                                     Act.Sigmoid)
                                sm2 = pc.tile([128, 512], dt.float32,
                                              tag="sm2", bufs=2)
                                nc.vector._custom_dve(
                                    BLEND2, out=sm2[:], in0=a2s,
                                    in1=p2g[:], s0=c3k_pp[:, 2:3],
                                    s1=c3k_pp[:, 0:1])
                                oc = pc.tile([128, 512], dt.float32,
                                             tag="oc", bufs=2)
                                nc.vector.scalar_tensor_tensor(
                                    oc[:], sm2[:], c3k_pp[:, 1:2],
                                    occ_all[:, u * 512:(u + 1) * 512],
                                    op0=Alu.mult, op1=Alu.mult)
                                weng = nc.sync if u % 2 == 0 else nc.scalar
                                weng.dma_start(
                                    dram_view(out,
                                              [[T, 2], [NS, Q], [1, 512]],
                                              ci * 2 * T + hh * 512),
                                    oc[:])

                if "B" not in phases:
                    pm2.release()
            if "C" not in phases:
                nc.sync.dma_start(
                    dram_view(out, [[NS, Q], [1, Q]], 0), revc[:])


def _get_program():
    global _compiled
    if _compiled is None:
        _compiled = _build_program()
    return _compiled


def _make_in_maps(voxel_logits, sem_prob_dense):
    vl = np.ascontiguousarray(
        np.asarray(voxel_logits, dtype=np.float32).reshape(S, Q, N))
    sp = np.ascontiguousarray(
        np.asarray(sem_prob_dense, dtype=np.float32).reshape(C_SEM, N))
    revcnt = np.tile((Q - np.arange(Q, dtype=np.float32))[None, :], (Q, 1))
    iotap = np.arange(128, dtype=np.float32)[:, None]
    in_maps = []
    for c in range(NCORES):
        sl = slice(c * NS, (c + 1) * NS)
        in_maps.append({
            "l0": np.ascontiguousarray(vl[0, :, sl]),
            "l1": np.ascontiguousarray(vl[1, :, sl]),
            "l2": np.ascontiguousarray(vl[2, :, sl]),
            "sem": np.ascontiguousarray(sp[:, sl]),
            "revcnt": revcnt,
            "iotap": iotap,
        })
    return in_maps


def profile_run(inputs):
    """Run once with NTFF tracing; returns exec_time_ns or None."""
    from concourse.bass_utils import run_bass_kernel_spmd

    nc = _get_program()
    in_maps = _make_in_maps(inputs["voxel_logits"], inputs["sem_prob_dense"])
    res = run_bass_kernel_spmd(nc, in_maps, list(range(NCORES)), trace=True)
    return res.exec_time_ns


def kernel(voxel_logits, query_logits, sem_prob_dense):
    from concourse.bass_utils import run_bass_kernel_spmd

    nc = _get_program()
    in_maps = _make_in_maps(voxel_logits, sem_prob_dense)
    res = run_bass_kernel_spmd(nc, in_maps, list(range(NCORES)))
    full = np.concatenate([res.results[c]["out"] for c in range(NCORES)],
                          axis=1)
    return full.reshape(Q, X, Y, Z).astype(np.float32)



# revision 3
# speedup vs baseline: 1.0221x; 1.0221x over previous
"""Trainium2 Bass kernel for nn_Ensembler (nms_detection).

Contract: kernel(**inputs) takes the FULL unsharded inputs
(voxel_logits [3,64,128,128,32] f32, query_logits [3,1,64,21] f32,
sem_prob_dense [21,128,128,32] f32) and returns the FULL output
[64,128,128,32] f32.

Strategy: shard the voxel grids over the flattened voxel dimension
N = X*Y*Z across 8 NeuronCores (each core owns a contiguous slice of
N).  The QxQ IoU statistics are computed as per-shard 0/1-mask GEMMs
(fp8 on the tensor engine) reduced with a tiny AllReduce; the
argmax / matching / merge / keep steps are then replicated on every
core, and the merge + keep + occupancy masking are embarrassingly
parallel over the local N slice.

v2: the data-dependent row gathers aux_v[aux_idx] are realized as
indirect DMAs (SWDGE row gather with device-computed indices) instead
of one-hot fp32 matmuls on the PE — this removes ~220us of PE-bound
critical path.  The per-core q-layout is [128 part = (qb, q),
H = NS/2 cols] with n = qb*H + j, so each partition's columns are a
contiguous half-row in DRAM and a single indirect DMA with
idx2 = 2*aux_idx + qb and coef H gathers a full [128, W] window.

Numerical notes:
 - all mask decisions are computed from logit signs (exact): the
   iteration-2 anchor mask uses (sig(x0)+sig(x1))/2 > 0.5 <=>
   x0 + x1 > 0, avoiding sigmoid-LUT error in the decision path.
 - sigmoid LUT (ScalarE) max abs err ~3.6e-6 affects output values
   only.
"""

import numpy as np

S = 3
Q = 64
X, Y, Z = 128, 128, 32
N = X * Y * Z           # 524288
C_SEM = 21
NCORES = 8
NS = N // NCORES        # 65536 voxels per core
H = NS // 2             # 32768 cols per partition in q-layout
JP = NS // 128          # 512 contiguous voxels per partition (n-layout)
QC = 4                  # q rows per n-layout read chunk
UC = 1024               # blend unit cols
NU = H // UC            # 32 blend units
GT = 2048               # gather tile cols
NG = H // GT            # 16 gather DMAs per pass
LB = 4096               # l0q tile cols (8 tiles)

_compiled = None


def _register_custom_dve_ops():
    """Register two fused DVE ops at runtime (halves the DVE op count on
    the blend/mask hot paths).  Purely additive registration in the
    concourse dve_ops tables; rows stay within the 5-bit byte-36 field."""
    import concourse.dve_ops as dve_ops
    from concourse.dve_ops import DveOp
    from concourse.dve_spec import (Spec, Src0, Src1, C0, C1, Zero, lower,
                                    _has_src1)
    from concourse.dve_uop import DveOpSpec

    if "ANT_BLEND2_K" in dve_ops._SUB_OPCODE_FOR_NAME:
        by = {op.name: op for op in dve_ops.OPS}
        return by["ANT_BLEND2_K"], by["ANT_MASKGT_K"]

    def make(name, spec):
        row = dve_ops._CUSTOM_DVE_ROW_BASE + len(dve_ops.OPS)
        assert row < 0x20
        dve_ops._SUB_OPCODE_FOR_NAME[name] = row
        shas = {}
        for ver in ("v3", "v4"):
            try:
                uops = lower(spec, ver=ver)
                shas[ver] = DveOpSpec(name=name, opcode=row, uops=uops,
                                      rd1_en=_has_src1(spec)).sha(ver)
            except Exception:
                pass
        op = DveOp(name, spec, subdim=False, uops_sha=shas)
        dve_ops.OPS.append(op)
        dve_ops.CUSTOM_DVE_SPECS[name] = spec
        return op

    blend2 = make("ANT_BLEND2_K", Spec(
        body=Src0 * C0 + Src1 * C1,
        reference=lambda in0, in1, s0, s1, imm2: (
            in0.astype(np.float32) * s0 + in1 * s1).astype(np.float32),
    ))
    maskgt = make("ANT_MASKGT_K", Spec(
        body=Zero < (Src0 + Src1 * C0),
        reference=lambda in0, in1, s0, s1, imm2: (
            (in0.astype(np.float32) + in1 * s0) > 0).astype(np.float32),
    ))
    return blend2, maskgt


def _build_program(phases=("A", "AR1", "B", "G2", "AR2", "C"), real_cc=True,
                   loop_k=None):
    import dataclasses
    import concourse.bass as bass
    import concourse.bacc as bacc
    import concourse.mybir as mybir
    import concourse.tile as tile

    phases = set(phases)
    dt = mybir.dt

    BLEND2, MASKGT = _register_custom_dve_ops()

    def dram_view(ap, pattern, offset_elems):
        """Raw [step,count] (element units) view of a DRAM tensor AP."""
        return dataclasses.replace(ap, ap=[list(p) for p in pattern],
                                   offset=offset_elems)

    nc = bacc.Bacc("TRN2", target_bir_lowering=False, debug=False,
                   num_devices=NCORES)

    l0 = nc.dram_tensor("l0", [Q, NS], dt.float32, kind="ExternalInput").ap()
    l1 = nc.dram_tensor("l1", [Q, NS], dt.float32, kind="ExternalInput").ap()
    l2 = nc.dram_tensor("l2", [Q, NS], dt.float32, kind="ExternalInput").ap()
    sem = nc.dram_tensor("sem", [C_SEM, NS], dt.float32,
                         kind="ExternalInput").ap()
    revcnt = nc.dram_tensor("revcnt", [Q, Q], dt.float32,
                            kind="ExternalInput").ap()
    iotap = nc.dram_tensor("iotap", [128, 1], dt.float32,
                           kind="ExternalInput").ap()
    out = nc.dram_tensor("out", [Q, NS], dt.float32,
                         kind="ExternalOutput").ap()

    import contextlib

    with tile.TileContext(nc) as tc:
        with (tc.For_i(0, loop_k, 1) if loop_k else
              contextlib.nullcontext()):
            _body(nc, tc, phases, real_cc, dram_view,
                  (l0, l1, l2, sem, revcnt, iotap, out), (BLEND2, MASKGT),
                  mybir, bass)
    nc.compile()
    return nc


def _body(nc, tc, phases, real_cc, dram_view, tensors, custom_ops, mybir,
          bass):
    import dataclasses

    dt = mybir.dt
    Alu = mybir.AluOpType
    Act = mybir.ActivationFunctionType
    l0, l1, l2, sem, revcnt, iotap, out = tensors
    BLEND2, MASKGT = custom_ops

    if True:
        with tc.tile_pool(name="dram", bufs=1, space="DRAM") as dramp, \
             tc.tile_pool(name="psum", bufs=1, space="PSUM") as psump, \
             tc.tile_pool(name="stats", bufs=1) as stp:

            # ---- DRAM scratch ----------------------------------------
            m0_dram = dramp.tile([Q + 1, NS], dt.float8e4)
            ma2_dram = dramp.tile([Q + 1, NS], dt.float8e4)
            occ_dram = dramp.tile([1, NS], dt.float8e4)
            cc_in1 = dramp.tile([Q + 1, Q + 1], dt.float32)
            cc_out1 = dramp.tile([Q + 1, Q + 1], dt.float32)
            cc_in2 = dramp.tile([Q + 1, Q + 1], dt.float32)
            cc_out2 = dramp.tile([Q + 1, Q + 1], dt.float32)
            pack1_dram = dramp.tile([Q, 3], dt.float32)
            pack2_dram = dramp.tile([Q, 3], dt.float32)
            idx1_dram = dramp.tile([1, Q], dt.float32)
            idx2_dram = dramp.tile([1, Q], dt.float32)

            # ---- small persistent stat tiles -------------------------
            revc = stp.tile([Q, Q], dt.float32)
            nc.sync.dma_start(revc[:], revcnt[:])
            iou_a1 = stp.tile([Q, 1], dt.float32)
            iou_a2 = stp.tile([Q, 1], dt.float32)
            iotp = stp.tile([128, 1], dt.float32)
            nc.sync.dma_start(iotp[:], iotap[:])
            qbv = stp.tile([128, 1], dt.float32)   # 0 for p<64, 1 for p>=64
            nc.vector.tensor_scalar(qbv[:], iotp[:], 63.5, None,
                                    op0=Alu.is_gt)
            cb_pp = stp.tile([128, 3], dt.float32)   # [cb, matched1, 1-cb]
            c3k_pp = stp.tile([128, 3], dt.float32)  # [c3, keep, 1-c3]
            gidx1 = stp.tile([128, 1], dt.int32)     # 2*aux_idx1 + qb
            gidx2 = stp.tile([128, 1], dt.int32)     # 2*aux_idx2 + qb

            g1_ps = psump.tile([Q + 1, Q + 1], dt.float32)
            g2_ps = psump.tile([Q + 1, Q + 1], dt.float32)

            # indirect-gather DRAM views: [2Q, H] row-contiguous
            l1g_view = dram_view(l1, [[H, 2 * Q], [1, H]], 0)
            l2g_view = dram_view(l2, [[H, 2 * Q], [1, H]], 0)

            # big persistent region: holds L0 logits (q-layout), then
            # anchor2 in place.  8 tiles of LB cols each.
            with tc.tile_pool(name="bigp", bufs=1) as bigp:
                l0q_tiles = []
                for b in range(8):
                    lt = bigp.tile([128, LB], dt.float32, name=f"l0q_{b}")
                    l0q_tiles.append(lt)
                    for qb in range(2):
                        eng = nc.sync if (b + qb) % 2 == 0 else nc.scalar
                        eng.dma_start(
                            lt[qb * Q:(qb + 1) * Q, :],
                            dram_view(l0, [[NS, Q], [1, LB]],
                                      qb * H + b * LB))

                def l0q_slice(u):
                    # unit u covers q-layout cols [u*UC, (u+1)*UC)
                    ti, off = divmod(u * UC, LB)
                    return l0q_tiles[ti][:, off:off + UC]

                # =====================================================
                # PASS A: m0 masks -> DRAM roundtrip (layout switch);
                #         m1 (SBUF n-layout) -> G1; m2 kept for G2
                # =====================================================
                with tc.tile_pool(name="m0p", bufs=1) as pa:
                    ones_c = pa.tile([128, JP], dt.float8e4)
                    nc.vector.memset(ones_c[:], 1.0)
                    nc.scalar.dma_start(
                        dram_view(m0_dram, [[JP, 128], [1, JP]], Q * NS),
                        ones_c[:])
                    # m0 masks from the q-layout L0 tiles -> m0_dram
                    for b in range(8):
                        m0c = pa.tile([128, LB], dt.float8e4, tag="m0c",
                                      bufs=2)
                        nc.vector.tensor_scalar(
                            m0c[:], l0q_tiles[b][:], 0.0, None,
                            op0=Alu.is_gt)
                        for qb in range(2):
                            weng = nc.scalar if (b + qb) % 2 == 0 else nc.sync
                            weng.dma_start(
                                dram_view(m0_dram, [[NS, Q], [1, LB]],
                                          qb * H + b * LB),
                                m0c[qb * Q:(qb + 1) * Q, :])
                    # m1 masks: n-layout direct to SBUF (j-major + ones col)
                    with tc.tile_pool(name="m1p", bufs=1) as pm1:
                        m1_sb = pm1.tile([128, JP, Q + 1], dt.float8e4)
                        nc.vector.memset(m1_sb[:, :, Q], 1.0)
                        for qc in range(Q // QC):
                            lc = pm1.tile([128, QC, JP], dt.float32,
                                          tag="ldchunk", bufs=2)
                            src = dram_view(l1,
                                            [[JP, 128], [NS, QC], [1, JP]],
                                            qc * QC * NS)
                            ldeng = nc.sync if qc % 2 == 0 else nc.scalar
                            ldeng.dma_start(lc[:], src)
                            nc.vector.tensor_scalar(
                                m1_sb[:, :, qc * QC:(qc + 1) * QC],
                                lc[:].rearrange("p q j -> p j q"), 0.0,
                                None, op0=Alu.is_gt)
                        # G1 GEMM: m0 readback (j-halves) x m1_sb
                        for h in range(2):
                            m0t = pm1.tile([128, Q + 1, JP // 2],
                                           dt.float8e4, tag="m0t", bufs=1)
                            nc.sync.dma_start(
                                m0t[:],
                                dram_view(
                                    m0_dram,
                                    [[JP, 128], [NS, Q + 1], [1, JP // 2]],
                                    h * (JP // 2)))
                            for j in range(JP // 2):
                                gj = h * (JP // 2) + j
                                nc.tensor.matmul(
                                    g1_ps[:], lhsT=m0t[:, :, j],
                                    rhs=m1_sb[:, gj, :],
                                    start=(gj == 0), stop=(gj == JP - 1))

                # m2 masks: n-layout direct to SBUF, persists through G2
                pm2 = tc.alloc_tile_pool(name="m2p", bufs=1)
                m2_sb = pm2.tile([128, JP, Q + 1], dt.float8e4)
                nc.vector.memset(m2_sb[:, :, Q], 1.0)
                with tc.tile_pool(name="m2fill", bufs=1) as pmf:
                    for qc in range(Q // QC):
                        lc2 = pmf.tile([128, QC, JP], dt.float32,
                                       tag="ld2chunk", bufs=2)
                        src = dram_view(l2, [[JP, 128], [NS, QC], [1, JP]],
                                        qc * QC * NS)
                        ldeng = nc.scalar if qc % 2 == 0 else nc.sync
                        ldeng.dma_start(lc2[:], src)
                        nc.vector.tensor_scalar(
                            m2_sb[:, :, qc * QC:(qc + 1) * QC],
                            lc2[:].rearrange("p q j -> p j q"), 0.0,
                            None, op0=Alu.is_gt)

                # ---- shared stats machinery --------------------------
                def stats_round(g_ps, cc_in, cc_out, iou_a, idx_dram, gidx):
                    sfx = cc_in.name
                    gs = stp.tile([Q + 1, Q + 1], dt.float32,
                                  name=f"gs_{sfx}")
                    nc.vector.tensor_copy(gs[:], g_ps[:])
                    nc.sync.dma_start(cc_in[:], gs[:])
                    if real_cc:
                        nc.gpsimd.collective_compute(
                            "AllReduce", Alu.add,
                            replica_groups=[list(range(NCORES))],
                            ins=[cc_in.opt()], outs=[cc_out.opt()])
                    else:
                        nc.sync.dma_start(cc_out[:], cc_in[:])
                    gr = stp.tile([Q + 1, Q + 1], dt.float32,
                                  name=f"gr_{sfx}")
                    nc.sync.dma_start(gr[:], cc_out[:])
                    sbb = stp.tile([Q, Q], dt.float32, name=f"sbb_{sfx}")
                    row = cc_out[Q:Q + 1, 0:Q]
                    nc.sync.dma_start(
                        sbb[:], dataclasses.replace(
                            row, ap=[[0, Q]] + [list(p) for p in row.ap[1:]]))
                    inter = gr[0:Q, 0:Q]
                    sa = gr[0:Q, Q:Q + 1]
                    u = stp.tile([Q, Q], dt.float32, name=f"u_{sfx}")
                    nc.vector.tensor_scalar(u[:], inter, sa, None,
                                            op0=Alu.subtract)
                    nc.vector.tensor_tensor(u[:], sbb[:], u[:],
                                            op=Alu.subtract)
                    nc.vector.tensor_scalar(u[:], u[:], 1.0, None,
                                            op0=Alu.max)
                    nc.vector.reciprocal(u[:], u[:])
                    iou = stp.tile([Q, Q], dt.float32, name=f"iou_{sfx}")
                    nc.vector.tensor_tensor(iou[:], inter, u[:], op=Alu.mult)
                    nc.vector.tensor_reduce(iou_a[:], iou[:],
                                            axis=mybir.AxisListType.X,
                                            op=Alu.max)
                    matched = stp.tile([Q, 1], dt.float32, name=f"mt_{sfx}")
                    nc.vector.tensor_scalar(matched[:], iou_a[:], 0.2, None,
                                            op0=Alu.is_gt)
                    eq = stp.tile([Q, Q], dt.float32, name=f"eq_{sfx}")
                    nc.vector.tensor_scalar(eq[:], iou[:], iou_a[:, 0:1],
                                            None, op0=Alu.is_equal)
                    nc.vector.tensor_tensor(eq[:], eq[:], revc[:],
                                            op=Alu.mult)
                    sm = stp.tile([Q, 1], dt.float32, name=f"sm_{sfx}")
                    nc.vector.tensor_reduce(sm[:], eq[:],
                                            axis=mybir.AxisListType.X,
                                            op=Alu.max)
                    nc.vector.tensor_scalar(sm[:], sm[:], -1.0, float(Q),
                                            op0=Alu.mult, op1=Alu.add)
                    # gather indices: gidx[p] = 2*sm[p%64] + (p>=64)
                    nc.sync.dma_start(
                        dram_view(idx_dram, [[1, Q], [1, 1]], 0),
                        sm[:, 0:1])
                    rep = stp.tile([128, 1], dt.float32,
                                   name=f"rep_{sfx}")
                    nc.sync.dma_start(
                        rep[:], dram_view(idx_dram, [[0, 2], [1, Q]], 0))
                    repi = stp.tile([128, 1], dt.float32,
                                    name=f"repi_{sfx}")
                    nc.vector.scalar_tensor_tensor(
                        repi[:], rep[:], 2.0, qbv[:],
                        op0=Alu.mult, op1=Alu.add)
                    nc.vector.tensor_copy(gidx[:], repi[:])
                    return matched

                if "AR1" in phases:
                    matched1 = stats_round(g1_ps, cc_in1, cc_out1, iou_a1,
                                           idx1_dram, gidx1)
                    cb64 = stp.tile([Q, 3], dt.float32)
                    nc.vector.tensor_scalar(cb64[:, 0:1], matched1[:], 0.5,
                                            None, op0=Alu.mult)
                    nc.vector.tensor_copy(cb64[:, 1:2], matched1[:])
                    nc.vector.tensor_scalar(cb64[:, 2:3], matched1[:], -0.5,
                                            1.0, op0=Alu.mult, op1=Alu.add)
                    nc.sync.dma_start(pack1_dram[:], cb64[:])
                    nc.sync.dma_start(
                        cb_pp[:],
                        dram_view(pack1_dram, [[0, 2], [3, Q], [1, 3]], 0))

                # =====================================================
                # PASS B: indirect gather of l1 rows; anchor2 blend in
                #         place + ma2 mask -> DRAM; G2 GEMM
                # =====================================================
                if "B" in phases:
                    with tc.tile_pool(name="blend", bufs=1) as pb:
                        ones_r = pb.tile([128, JP], dt.float8e4)
                        nc.vector.memset(ones_r[:], 1.0)
                        nc.scalar.dma_start(
                            dram_view(ma2_dram, [[JP, 128], [1, JP]],
                                      Q * NS),
                            ones_r[:])
                        for g in range(NG):
                            lgt = pb.tile([128, GT], dt.float32, tag="lgt",
                                          bufs=2)
                            nc.gpsimd.indirect_dma_start(
                                out=lgt[:], out_offset=None,
                                in_=l1g_view,
                                in_offset=bass.IndirectOffsetOnAxis(
                                    ap=gidx1[:, :1], axis=0),
                                element_offset=g * GT)
                            for hh in range(GT // UC):
                                u = g * (GT // UC) + hh
                                sl = l0q_slice(u)
                                lgu = lgt[:, hh * UC:(hh + 1) * UC]
                                ma2u = pb.tile([128, UC], dt.float8e4,
                                               tag="ma2u", bufs=2)
                                # exact mask (l0 + matched1*l1g) > 0
                                nc.vector._custom_dve(
                                    MASKGT, out=ma2u[:], in0=sl, in1=lgu,
                                    s0=cb_pp[:, 1:2])
                                for qb in range(2):
                                    weng = (nc.scalar if (u + qb) % 2 == 0
                                            else nc.sync)
                                    weng.dma_start(
                                        dram_view(ma2_dram,
                                                  [[NS, Q], [1, UC]],
                                                  qb * H + u * UC),
                                        ma2u[qb * Q:(qb + 1) * Q, :])
                                p0c = pb.tile([128, UC], dt.float32,
                                              tag="p0c", bufs=2)
                                nc.scalar.activation(p0c[:], sl, Act.Sigmoid)
                                p1g = pb.tile([128, UC], dt.float32,
                                              tag="p1g", bufs=2)
                                nc.scalar.activation(p1g[:], lgu,
                                                     Act.Sigmoid)
                                # anchor2 = (1-cb)*p0 + cb*p1g, in place
                                nc.vector._custom_dve(
                                    BLEND2, out=sl, in0=p0c[:], in1=p1g[:],
                                    s0=cb_pp[:, 2:3], s1=cb_pp[:, 0:1])

                    if "G2" in phases:
                        with tc.tile_pool(name="g2", bufs=1) as pg:
                            ma2t = pg.tile([128, Q + 1, JP], dt.float8e4)
                            for g in range(8):
                                ps = slice(g * 16, (g + 1) * 16)
                                eng = nc.sync if g % 2 == 0 else nc.scalar
                                eng.dma_start(
                                    ma2t[ps, :, :],
                                    dram_view(
                                        ma2_dram,
                                        [[JP, 16], [NS, Q + 1], [1, JP]],
                                        g * 16 * JP))
                            for j in range(JP):
                                nc.tensor.matmul(
                                    g2_ps[:], lhsT=ma2t[:, :, j],
                                    rhs=m2_sb[:, j, :],
                                    start=(j == 0), stop=(j == JP - 1))
                    pm2.release()

                    # occupancy: independent of AR2 -> fills its window
                    # occ[n] = (max_{c>=1} sem[c,n] > sem[0,n])
                    if "C" in phases:
                        with tc.tile_pool(name="occp", bufs=1) as po:
                            sem0 = po.tile([128, JP], dt.float32)
                            nc.sync.dma_start(
                                sem0[:],
                                dram_view(sem, [[JP, 128], [1, JP]], 0))
                            mx = po.tile([128, JP], dt.float32)
                            nc.sync.dma_start(
                                mx[:],
                                dram_view(sem, [[JP, 128], [1, JP]], NS))
                            for g0 in range(2, C_SEM, 5):
                                rows = min(5, C_SEM - g0)
                                semc = po.tile([128, 5, JP], dt.float32,
                                               tag="semc", bufs=1,
                                               name=f"semg{g0}")
                                nc.scalar.dma_start(
                                    semc[:, :rows, :],
                                    dram_view(sem,
                                              [[JP, 128], [NS, rows],
                                               [1, JP]],
                                              g0 * NS))
                                for k in range(rows):
                                    nc.vector.tensor_tensor(
                                        mx[:], mx[:], semc[:, k, :],
                                        op=Alu.max)
                            occ_n = po.tile([128, JP], dt.float8e4)
                            nc.vector.tensor_tensor(occ_n[:], mx[:],
                                                    sem0[:], op=Alu.is_gt)
                            nc.sync.dma_start(
                                dram_view(occ_dram, [[JP, 128], [1, JP]],
                                          0),
                                occ_n[:])

                    if "AR2" in phases:
                        matched2 = stats_round(g2_ps, cc_in2, cc_out2,
                                               iou_a2, idx2_dram, gidx2)
                        pk = stp.tile([Q, 3], dt.float32)
                        nc.vector.tensor_scalar(pk[:, 0:1], matched2[:],
                                                1.0 / 3.0, None,
                                                op0=Alu.mult)
                        nc.vector.tensor_scalar(pk[:, 2:3], matched2[:],
                                                -1.0 / 3.0, 1.0,
                                                op0=Alu.mult, op1=Alu.add)
                        t64 = stp.tile([Q, 1], dt.float32)
                        nc.vector.tensor_tensor(t64[:], iou_a1[:],
                                                iou_a2[:], op=Alu.add)
                        nc.vector.tensor_scalar(pk[:, 1:2], t64[:], 0.5,
                                                0.2, op0=Alu.mult,
                                                op1=Alu.is_gt)
                        nc.sync.dma_start(pack2_dram[:], pk[:])
                        nc.sync.dma_start(
                            c3k_pp[:],
                            dram_view(pack2_dram, [[0, 2], [3, Q], [1, 3]],
                                      0))

                    # =================================================
                    # PASS C: indirect gather of l2 rows; final merge +
                    #         keep + occupancy -> out
                    # =================================================
                    if "C" in phases:
                        with tc.tile_pool(name="passc", bufs=1) as pc:
                            for g in range(NG):
                                lgt2 = pc.tile([128, GT], dt.float32,
                                               tag="lgt2", bufs=2)
                                nc.gpsimd.indirect_dma_start(
                                    out=lgt2[:], out_offset=None,
                                    in_=l2g_view,
                                    in_offset=bass.IndirectOffsetOnAxis(
                                        ap=gidx2[:, :1], axis=0),
                                    element_offset=g * GT)
                                occu = pc.tile([128, GT], dt.float8e4,
                                               tag="occu", bufs=2)
                                oeng = nc.sync if g % 2 == 0 else nc.scalar
                                oeng.dma_start(
                                    occu[:],
                                    dram_view(occ_dram,
                                              [[H, 2], [0, Q], [1, GT]],
                                              g * GT))
                                for hh in range(GT // UC):
                                    u = g * (GT // UC) + hh
                                    a2s = l0q_slice(u)
                                    lgu = lgt2[:, hh * UC:(hh + 1) * UC]
                                    p2g = pc.tile([128, UC], dt.float32,
                                                  tag="p2g", bufs=2)
                                    nc.scalar.activation(p2g[:], lgu,
                                                         Act.Sigmoid)
                                    sm2 = pc.tile([128, UC], dt.float32,
                                                  tag="sm2", bufs=2)
                                    nc.vector._custom_dve(
                                        BLEND2, out=sm2[:], in0=a2s,
                                        in1=p2g[:], s0=c3k_pp[:, 2:3],
                                        s1=c3k_pp[:, 0:1])
                                    oc = pc.tile([128, UC], dt.float32,
                                                 tag="oc", bufs=2)
                                    nc.vector.scalar_tensor_tensor(
                                        oc[:], sm2[:], c3k_pp[:, 1:2],
                                        occu[:, hh * UC:(hh + 1) * UC],
                                        op0=Alu.mult, op1=Alu.mult)
                                    for qb in range(2):
                                        weng = (nc.sync if (u + qb) % 2 == 0
                                                else nc.scalar)
                                        weng.dma_start(
                                            dram_view(out,
                                                      [[NS, Q], [1, UC]],
                                                      qb * H + u * UC),
                                            oc[qb * Q:(qb + 1) * Q, :])

                if "B" not in phases:
                    pm2.release()
            if "C" not in phases:
                nc.sync.dma_start(
                    dram_view(out, [[NS, Q], [1, Q]], 0), revc[:])


def _get_program():
    global _compiled
    if _compiled is None:
        _compiled = _build_program()
    return _compiled


def _make_in_maps(voxel_logits, sem_prob_dense):
    vl = np.ascontiguousarray(
        np.asarray(voxel_logits, dtype=np.float32).reshape(S, Q, N))
    sp = np.ascontiguousarray(
        np.asarray(sem_prob_dense, dtype=np.float32).reshape(C_SEM, N))
    revcnt = np.tile((Q - np.arange(Q, dtype=np.float32))[None, :], (Q, 1))
    iotap = np.arange(128, dtype=np.float32)[:, None]
    in_maps = []
    for c in range(NCORES):
        sl = slice(c * NS, (c + 1) * NS)
        in_maps.append({
            "l0": np.ascontiguousarray(vl[0, :, sl]),
            "l1": np.ascontiguousarray(vl[1, :, sl]),
            "l2": np.ascontiguousarray(vl[2, :, sl]),
            "sem": np.ascontiguousarray(sp[:, sl]),
            "revcnt": revcnt,
            "iotap": iotap,
        })
    return in_maps


def profile_run(inputs):
    """Run once with NTFF tracing; returns exec_time_ns or None."""
    from concourse.bass_utils import run_bass_kernel_spmd

    nc = _get_program()
    in_maps = _make_in_maps(inputs["voxel_logits"], inputs["sem_prob_dense"])
    res = run_bass_kernel_spmd(nc, in_maps, list(range(NCORES)), trace=True)
    return res.exec_time_ns


def kernel(voxel_logits, query_logits, sem_prob_dense):
    from concourse.bass_utils import run_bass_kernel_spmd

    nc = _get_program()
    in_maps = _make_in_maps(voxel_logits, sem_prob_dense)
    res = run_bass_kernel_spmd(nc, in_maps, list(range(NCORES)))
    full = np.concatenate([res.results[c]["out"] for c in range(NCORES)],
                          axis=1)
    return full.reshape(Q, X, Y, Z).astype(np.float32)
